# revision 1
# baseline (speedup 1.0000x reference)
"""CLUBMean loss kernel for Trainium2, 8-core data-parallel.

Math: with x_vec = mean_s(x), y_vec = mean_s(y), mu = MLP(x_vec):
  positive_i = -||mu_i - y_i||^2 / 2
  negative_i = -mean_j ||y_j - mu_i||^2 / 2
             = -(S2/N - 2 mu_i . Ey + ||mu_i||^2) / 2      (exact expansion)
  loss = mean_i(positive_i - negative_i)

Each core handles 128 of the 1024 samples and emits partial sums:
  out_vec (128,4): columns [Ey lo, Ey hi, Mu lo, Mu hi] summed over its samples
  out_row (1,2,3,128): per-sample [||mu-y||^2, ||y||^2, ||mu||^2] by D-half
The host all-reduces the partials in float64 and applies the closed form.

Pipeline per core:
  - one HWDGE (sync) queue streams 24 x 1MiB channel chunks (first chunk split
    in half so pooling starts earlier); packed weights ride the gpsimd SWDGE
    queue in parallel
  - spatial pooling is split between DVE (tensor_reduce) and GPSIMD
    (tensor_add folds 64 spatial positions to 32, halving DVE work); folds
    cover early/mid chunks so the stream tail is low-latency direct reduces
  - PE transposes pooled x vectors to channel-major and runs the MLP as fp32
    matmuls accumulated in PSUM (fp32 accumulation groups must stay
    contiguous; interleaving groups miscompiles)
  - epilogue is split per D-half; the ||mu||^2 partition-sums run early so the
    tail only carries one N=256 ones-matmul per half

Each DMA's +16 semaphore arrives as +1 per DGE lane, so chunk completion uses
one semaphore per transfer (cumulative thresholds across chunks are unsound).
"""

import sys

sys.path.insert(0, "/opt/trn_rl_repo")

from contextlib import ExitStack

import numpy as np

import concourse.bass as bass
import concourse.mybir as mybir
from concourse.bass_utils import run_bass_kernel_spmd
from concourse.masks import make_identity

N = 1024
P = 128            # samples per core
XC, YC, HID, S = 512, 256, 512, 64
CH = 32            # channel chunk per streamed DMA (1 MiB)
NBUF = 16          # stream buffer ring
NXV = 4            # pooled-vector ring
NF = 4             # fold buffer ring
WCOLS = 3584       # wpack padded to 14336 B/partition (512 B aligned)
F32 = mybir.dt.float32
AX = mybir.AxisListType
ALU = mybir.AluOpType
ACTF = mybir.ActivationFunctionType

# chunk table: (is_y, c0). x: 16 x 32ch; y: 8 x 32ch. All 32 wide
# (engine partition offsets must be 32-aligned).
CHUNKS = [(0, c * CH) for c in range(16)] + [(1, c * CH) for c in range(8)]
NCHUNK = len(CHUNKS)   # 24
NX = 16                # x chunks
SPLIT = {0}            # chunks whose DMA is split in half (earlier start)
# chunks folded 64->32 spatially by GPSIMD; early/mid so the stream tail is
# handled by low-latency direct DVE reduces
FOLD = {1, 2, 4, 5, 6, 7, 8, 9, 10, 11, 12, 13, 14, 16, 17, 18}

_CACHE = {}


def build_nc(debug=False):
    nc = bass.Bass()
    x = nc.dram_tensor("x", [P, XC, S], F32, kind="ExternalInput")
    y = nc.dram_tensor("y", [P, YC, S], F32, kind="ExternalInput")
    # all weights packed host-side into final SBUF layout:
    # [w1 (4k x 512h) | w2 (4k x 256c) | b1 (4) | b2 (2) | pad] per partition
    wpack = nc.dram_tensor("wpack", [P, WCOLS], F32, kind="ExternalInput")
    out_vec = nc.dram_tensor("out_vec", [P, 4], F32, kind="ExternalOutput")
    out_row = nc.dram_tensor("out_row", [1, 2, 3, P], F32, kind="ExternalOutput")
    if debug:
        dbg_xvT = nc.dram_tensor("dbg_xvT", [P, 4, P], F32, kind="ExternalOutput")
        dbg_hT = nc.dram_tensor("dbg_hT", [P, 4, P], F32, kind="ExternalOutput")
        dbg_muT = nc.dram_tensor("dbg_muT", [P, 2, P], F32, kind="ExternalOutput")

    ctx = ExitStack()
    with ctx:
        sb = lambda name, shape: ctx.enter_context(nc.sbuf_tensor(name, shape, F32))
        ps = lambda name, shape: ctx.enter_context(nc.psum_tensor(name, shape, F32))
        sem = lambda name: ctx.enter_context(nc.semaphore(name))

        xbuf = [sb(f"xbuf{i}", [P, CH, S]) for i in range(NBUF)]
        fbuf = [sb(f"fbuf{i}", [P, CH, S // 2]) for i in range(NF)]
        xv = [sb(f"xv{i}", [P, CH]) for i in range(NXV)]
        xvT = sb("xvT", [P, 4, P])
        yvT = sb("yvT", [P, 2, P])
        hT = sb("hT", [P, 4, P])
        muT = sb("muT", [P, 2, P])
        dtmp = sb("dtmp", [P, P])
        # [:, m, 0..2, :] = sqd, sqy, sqmu (sqd/sqy adjacent: one tail matmul)
        sq_all = sb("sq_all", [P, 2, 3, P])
        wsb = sb("wsb", [P, WCOLS])
        ident = sb("ident", [P, P])
        ones = sb("ones", [P, 1])
        stat = sb("stat", [P, 4])
        rows = sb("rows", [1, 2, 3, P])

        pt = [ps(f"pt{i}", [CH, P]) for i in range(2)]
        ph = ps("ph", [P, 4, P])
        pmu = ps("pmu", [P, 2, P])
        prow = [ps(f"prow{i}", [1, 3, P]) for i in range(2)]

        # per-transfer DMA table: (chunk, c_lo, c_hi) with its own sem
        DMAS = []
        for i in range(NCHUNK):
            if i in SPLIT:
                DMAS.append((i, 0, CH // 2))
                DMAS.append((i, CH // 2, CH))
            else:
                DMAS.append((i, 0, CH))
        dsem = {}
        for (i, lo, hi) in DMAS:
            dsem[(i, lo)] = sem(f"d{i}_{lo}")
        dw = sem("dw")
        dout = sem("dout")
        s_const = sem("s_const")
        s_pool = sem("s_pool")
        s_fold = sem("s_fold")
        s_tp = sem("s_tp")
        s_cp = sem("s_cp")
        s_hmm = sem("s_hmm")
        s_relu = sem("s_relu")
        s_mumm = sem("s_mumm")
        s_mubias = sem("s_mubias")
        s_musq = sem("s_musq")
        s_sq = [sem("s_sq0"), sem("s_sq1")]
        s_stat = sem("s_stat")
        s_row = [sem("s_row0"), sem("s_row1")]
        s_rowcp = [sem("s_rowcp0"), sem("s_rowcp1")]

        FOLD_LIST = sorted(FOLD)
        FOLD_RANK = {i: r for r, i in enumerate(FOLD_LIST)}

        def chunk_src(i, lo, hi):
            is_y, c0 = CHUNKS[i]
            t = y if is_y else x
            return t[:, c0 + lo:c0 + hi, :]

        def issue_dma(e, i, lo, hi):
            e.dma_start(
                out=xbuf[i % NBUF][:, lo:hi, :], in_=chunk_src(i, lo, hi)
            ).then_inc(dsem[(i, lo)], 16)

        def copy_dst(i):
            is_y, c0 = CHUNKS[i]
            t = yvT if is_y else xvT
            return t[c0 % P:c0 % P + CH, c0 // P, :]

        # vector helpers -------------------------------------------------
        def _early_mu_block(e):
            # everything that only needs muT: ||mu||^2 + Mu stats
            e.wait_ge(s_mubias, 2)
            e.tensor_mul(sq_all[:, 0, 2, :], muT[:, 0, :], muT[:, 0, :])
            e.tensor_mul(
                sq_all[:, 1, 2, :], muT[:, 1, :], muT[:, 1, :]
            ).then_inc(s_musq, 1)
            e.tensor_reduce(stat[:, 2:3], muT[:, 0, :], axis=AX.X, op=ALU.add)
            e.tensor_reduce(stat[:, 3:4], muT[:, 1, :], axis=AX.X, op=ALU.add)

        def _half_epilogue(e, m):
            e.wait_ge(s_mubias, 2)
            e.wait_ge(s_cp, 20 if m == 0 else NCHUNK)
            e.tensor_sub(dtmp[:, :], muT[:, m, :], yvT[:, m, :])
            e.tensor_mul(sq_all[:, m, 0, :], dtmp[:, :], dtmp[:, :])
            e.tensor_mul(
                sq_all[:, m, 1, :], yvT[:, m, :], yvT[:, m, :]
            ).then_inc(s_sq[m], 1)
            inst = e.tensor_reduce(
                stat[:, m:m + 1], yvT[:, m, :], axis=AX.X, op=ALU.add
            )
            if m == 1:
                inst.then_inc(s_stat, 1)

        with nc.Block() as block:

            @block.sync
            def _(e):
                ndma = 0
                for i, lo, hi in DMAS:
                    if ndma == 5:
                        # weights ride the stream after the first few chunks
                        e.dma_start(out=wsb[:, :], in_=wpack[:, :]).then_inc(
                            dw, 16
                        )
                    ndma += 1
                    if i >= NBUF:
                        # ring reuse guard: fold chunks free their buffer at
                        # the gpsimd fold, direct chunks at the DVE reduce
                        j = i - NBUF
                        if j in FOLD:
                            e.wait_ge(s_fold, FOLD_RANK[j] + 1)
                        else:
                            e.wait_ge(s_pool, j + 1)
                    issue_dma(e, i, lo, hi)
                e.wait_ge(s_rowcp[0], 1)
                e.dma_start(
                    out=out_row[:, 0, :, :], in_=rows[:, 0, :, :]
                ).then_inc(dout, 16)
                e.wait_ge(s_stat, 1)
                e.dma_start(out=out_vec[:, :], in_=stat[:, :]).then_inc(dout, 16)
                e.wait_ge(s_rowcp[1], 1)
                e.dma_start(
                    out=out_row[:, 1, :, :], in_=rows[:, 1, :, :]
                ).then_inc(dout, 16)
                if debug:
                    e.dma_start(out=dbg_xvT[:, :, :], in_=xvT[:, :, :]).then_inc(dout, 16)
                    e.dma_start(out=dbg_hT[:, :, :], in_=hT[:, :, :]).then_inc(dout, 16)
                    e.dma_start(out=dbg_muT[:, :, :], in_=muT[:, :, :]).then_inc(dout, 16)
                e.wait_ge(dout, 48 + (48 if debug else 0))

            @block.gpsimd
            def _(e):
                make_identity(nc, ident[:, :])
                e.memset(ones[:, :], 1.0).then_inc(s_const, 1)
                # spatial fold 64->32 for the FOLD chunks (halves DVE work)
                for r, i in enumerate(FOLD_LIST):
                    e.wait_ge(dsem[(i, 0)], 16)
                    if r >= NF:
                        # fbuf ring: the DVE reduce of fold r-NF must be done
                        e.wait_ge(s_pool, FOLD_LIST[r - NF] + 1)
                    e.tensor_add(
                        fbuf[r % NF][:, :, :],
                        xbuf[i % NBUF][:, :, 0:S // 2],
                        xbuf[i % NBUF][:, :, S // 2:S],
                    ).then_inc(s_fold, 1)

            @block.vector
            def _(e):
                for i in range(NCHUNK):
                    if i >= NXV:
                        e.wait_ge(s_tp, i - NXV + 1)
                    if i in FOLD:
                        e.wait_ge(s_fold, FOLD_RANK[i] + 1)
                        e.tensor_reduce(
                            xv[i % NXV][:, :],
                            fbuf[FOLD_RANK[i] % NF][:, :, :],
                            axis=AX.X,
                            op=ALU.add,
                        ).then_inc(s_pool, 1)
                    elif i in SPLIT:
                        h = CH // 2
                        e.wait_ge(dsem[(i, 0)], 16)
                        e.tensor_reduce(
                            xv[i % NXV][:, 0:h],
                            xbuf[i % NBUF][:, 0:h, :],
                            axis=AX.X,
                            op=ALU.add,
                        )
                        e.wait_ge(dsem[(i, h)], 16)
                        e.tensor_reduce(
                            xv[i % NXV][:, h:CH],
                            xbuf[i % NBUF][:, h:CH, :],
                            axis=AX.X,
                            op=ALU.add,
                        ).then_inc(s_pool, 1)
                    else:
                        e.wait_ge(dsem[(i, 0)], 16)
                        e.tensor_reduce(
                            xv[i % NXV][:, :],
                            xbuf[i % NBUF][:, :, :],
                            axis=AX.X,
                            op=ALU.add,
                        ).then_inc(s_pool, 1)
                    if i == 19:
                        _early_mu_block(e)
                    if i == 20:
                        _half_epilogue(e, 0)
                _half_epilogue(e, 1)

            @block.tensor
            def _(e):
                e.wait_ge(s_const, 1)
                for i in range(NCHUNK):
                    e.wait_ge(s_pool, i + 1)
                    if i >= 2:
                        e.wait_ge(s_cp, i - 1)
                    e.transpose(
                        pt[i % 2][:, :], xv[i % NXV][:, :], ident[:, :]
                    ).then_inc(s_tp, 1)
                    if i == NX - 1:
                        # h = x_vec @ W1: fp32 accumulation groups must stay
                        # contiguous (interleaving groups miscompiles)
                        e.wait_ge(s_cp, NX)
                        e.wait_ge(dw, 16)
                        for m in range(4):
                            for k in range(4):
                                mm = e.matmul(
                                    ph[:, m, :],
                                    wsb[:, k * 512 + m * P:
                                        k * 512 + (m + 1) * P],
                                    xvT[:, k, :],
                                    start=(k == 0),
                                    stop=(k == 3),
                                )
                        mm.then_inc(s_hmm, 1)
                    if i == NX:
                        e.wait_ge(s_relu, 4)
                        for m in range(2):
                            for k in range(4):
                                mm = e.matmul(
                                    pmu[:, m, :],
                                    wsb[:, 2048 + k * 256 + m * P:
                                        2048 + k * 256 + (m + 1) * P],
                                    hT[:, k, :],
                                    start=(k == 0),
                                    stop=(k == 3),
                                )
                        mm.then_inc(s_mumm, 1)
                    if i == 19:
                        # ||mu||^2 partition sums (both halves) done early
                        e.wait_ge(s_musq, 1)
                        e.matmul(prow[0][:, 2, :], ones[:, :],
                                 sq_all[:, 0, 2, :], start=True, stop=True)
                        e.matmul(prow[1][:, 2, :], ones[:, :],
                                 sq_all[:, 1, 2, :], start=True, stop=True)
                    if i == 21:
                        e.wait_ge(s_sq[0], 1)
                        e.matmul(
                            prow[0][:, 0:2, :],
                            ones[:, :],
                            sq_all[:, 0, 0:2, :],
                            start=True,
                            stop=True,
                        ).then_inc(s_row[0], 1)
                e.wait_ge(s_sq[1], 1)
                e.matmul(
                    prow[1][:, 0:2, :],
                    ones[:, :],
                    sq_all[:, 1, 0:2, :],
                    start=True,
                    stop=True,
                ).then_inc(s_row[1], 1)

            @block.scalar
            def _(e):
                for i in range(NCHUNK):
                    e.wait_ge(s_tp, i + 1)
                    # fold the 1/64 spatial mean into the transpose copy (exact)
                    e.activation(
                        copy_dst(i), pt[i % 2][:, :], ACTF.Copy, scale=1.0 / S
                    ).then_inc(s_cp, 1)
                    if i == NX - 1:
                        e.wait_ge(s_hmm, 1)
                        for m in range(4):
                            e.activation(
                                hT[:, m, :],
                                ph[:, m, :],
                                ACTF.Relu,
                                bias=wsb[:, 3072 + m:3073 + m],
                            ).then_inc(s_relu, 1)
                    if i == NX:
                        e.wait_ge(s_mumm, 1)
                        for m in range(2):
                            e.activation(
                                muT[:, m, :],
                                pmu[:, m, :],
                                ACTF.Identity,
                                bias=wsb[:, 3076 + m:3077 + m],
                            ).then_inc(s_mubias, 1)
                    if i == 22:
                        e.wait_ge(s_row[0], 1)
                        e.activation(
                            rows[:, 0, :, :], prow[0][:, :, :], ACTF.Copy
                        ).then_inc(s_rowcp[0], 1)
                e.wait_ge(s_row[1], 1)
                e.activation(
                    rows[:, 1, :, :], prow[1][:, :, :], ACTF.Copy
                ).then_inc(s_rowcp[1], 1)

    return nc


def _get_nc():
    if "nc" not in _CACHE:
        _CACHE["nc"] = build_nc()
    return _CACHE["nc"]


def make_in_maps(x_samples, y_samples, W1, b1, W2, b2):
    xs = np.ascontiguousarray(
        np.asarray(x_samples, np.float32).reshape(N, XC, S)
    )
    ys = np.ascontiguousarray(
        np.asarray(y_samples, np.float32).reshape(N, YC, S)
    )
    wp = np.zeros((P, WCOLS), np.float32)
    wp[:, :2048] = (
        np.asarray(W1, np.float32).reshape(4, P, HID).transpose(1, 0, 2).reshape(P, 2048)
    )
    wp[:, 2048:3072] = (
        np.asarray(W2, np.float32).reshape(4, P, YC).transpose(1, 0, 2).reshape(P, 1024)
    )
    wp[:, 3072:3076] = np.asarray(b1, np.float32).reshape(4, P).T
    wp[:, 3076:3078] = np.asarray(b2, np.float32).reshape(2, P).T
    wp = np.ascontiguousarray(wp)
    in_maps = []
    for c in range(8):
        in_maps.append(
            {
                "x": np.ascontiguousarray(xs[c * P:(c + 1) * P]),
                "y": np.ascontiguousarray(ys[c * P:(c + 1) * P]),
                "wpack": wp,
            }
        )
    return in_maps


def combine(results):
    A = B = S2 = 0.0
    EyN = np.zeros(YC, np.float64)
    MuN = np.zeros(YC, np.float64)
    for c in range(8):
        vec = results[c]["out_vec"].astype(np.float64)    # (128, 4)
        row = results[c]["out_row"].astype(np.float64)    # (1, 2, 3, 128)
        EyN += np.concatenate([vec[:, 0], vec[:, 1]])
        MuN += np.concatenate([vec[:, 2], vec[:, 3]])
        A += row[0, :, 0, :].sum()
        S2 += row[0, :, 1, :].sum()
        B += row[0, :, 2, :].sum()
    ey = EyN / N
    mu = MuN / N
    loss = -(A / N) / 2.0 + 0.5 * (S2 / N - 2.0 * float(mu @ ey) + B / N)
    return np.float32(loss)


def run(inputs, **kwargs):
    nc = _get_nc()
    in_maps = make_in_maps(**inputs)
    res = run_bass_kernel_spmd(nc, in_maps, core_ids=list(range(8)), **kwargs)
    return combine(res.results), res


def kernel(x_samples, y_samples, W1, b1, W2, b2):
    loss, _ = run(
        dict(
            x_samples=x_samples,
            y_samples=y_samples,
            W1=W1,
            b1=b1,
            W2=W2,
            b2=b2,
        )
    )
    return loss



# revision 12
# speedup vs baseline: 1.1450x; 1.1450x over previous
"""CLUBMean loss kernel for Trainium2, 8-core data-parallel, fp16 stream.

Math: the reference loss collapses exactly (the quadratic terms cancel):
  loss = mean_i mu_i . (y_i - mean_j y_j)
       = (1/N) sum_i mu_i.y_i  -  (sum_i mu_i / N) . (sum_j y_j / N)
so the kernel only needs pooled vectors, the MLP, one covariance dot per
sample, and the two mean vectors. Samples are streamed as fp16 (host cast):
halves HBM traffic; measured end-to-end rel err ~2e-3 vs the 2e-2 gate.

Each core handles 128 of the 1024 samples:
  - sync HWDGE streams x (16 x 32ch) then y (8 x 32ch) fp16 chunks; first and
    last chunks split in half so the pool pipeline starts early / drains fast;
    fp16 weights + f32 biases ride the same queue early
  - pooling = level-1 spatial fold 64->32 (DVE tensor_tensor at 2x fp16 rate,
    a subset on GpSimd to share load) + DVE tensor_reduce (1x) into f32
  - PE transposes pooled x (f32), ACT scale-copies (1/64) to fp16; MLP runs as
    fp16 matmuls into f32 PSUM; mu is back-transposed to sample-major
  - per-sample dot D_n = sum_c mu[n,c]*yv[n,c] via fused tensor_tensor_reduce
    chained over column blocks; Sum_n mu via DVE reduces on muT
  - outputs: yv (pooled y, unscaled; host sums for the y-mean), stat (D, Mu)

Host combine (f64): loss = sum(D)/64/N - (Mu/N).(sum(yv)/64/N).
Each DMA's +16 semaphore arrives as +1 per DGE lane; chunk completion uses
one semaphore per transfer.
"""

import sys

sys.path.insert(0, "/opt/trn_rl_repo")

from contextlib import ExitStack

import numpy as np

import concourse.bass as bass
import concourse.mybir as mybir
from concourse.bass_utils import run_bass_kernel_spmd
N = 1024
P = 128            # samples per core
XC, YC, HID, S = 512, 256, 512, 64
CH = 32            # channels per streamed chunk
NBUF = 16          # stream buffer ring (chunk-indexed)
NF = 8             # fold buffer ring (unit-indexed)
WCOLS = 3072       # fp16 weight pack: w1 (4k x 512h) | w2 (4k x 256c)
F32 = mybir.dt.float32
F16 = mybir.dt.float16
AX = mybir.AxisListType
ALU = mybir.AluOpType
ACTF = mybir.ActivationFunctionType

# ---- chunk / pool-unit tables ----------------------------------------------
# DMA chunks: (is_y, c0) with CH channels each. x: 16, y: 8.
CHUNKS = [(0, c * CH) for c in range(16)] + [(1, c * CH) for c in range(8)]
NCHUNK = len(CHUNKS)       # 24
SPLIT = {0, 23}            # chunks whose DMA (and pooling) is split in half
WPOS = 6                   # weights ride the stream after this many DMAs

# pool units: (chunk, lo, hi) channel sub-ranges, in stream order
UNITS = []
for c in range(NCHUNK):
    if c in SPLIT:
        UNITS.append((c, 0, CH // 2))
        UNITS.append((c, CH // 2, CH))
    else:
        UNITS.append((c, 0, CH))
NU = len(UNITS)            # 26
ULAST = {}                 # chunk -> its last unit index
for u, (c, lo, hi) in enumerate(UNITS):
    ULAST[c] = u
# units whose level-1 fold runs on GpSimd (spread over early/mid x chunks so
# the Q7 cores keep pace with the stream; DVE takes the rest)
GUNITS = [u for u, (c, lo, hi) in enumerate(UNITS) if c in (1, 3, 5, 7, 9, 11, 13, 15)]
GRANK = {u: r for r, u in enumerate(GUNITS)}

# y column block boundaries (in yv columns) for the dot + output DMAs
# block A: y chunks 16-19 (cols 0:128), B: 20-22 (128:224), C: 23 (224:256)
POOL_A = ULAST[19] + 1
POOL_B = ULAST[22] + 1
POOL_C = ULAST[23] + 1

USE_TTR = False            # fused tensor_tensor_reduce fails walrus codegen
DEBUG = False              # extra debug output DMAs

_CACHE = {}


def build_nc(debug=False):
    nc = bass.Bass()
    x = nc.dram_tensor("x", [P, XC, S], F16, kind="ExternalInput")
    y = nc.dram_tensor("y", [P, YC, S], F16, kind="ExternalInput")
    wpack = nc.dram_tensor("wpack", [P, WCOLS], F16, kind="ExternalInput")
    bias = nc.dram_tensor("bias", [P, 8], F32, kind="ExternalInput")
    ident_in = nc.dram_tensor("ident_in", [P, P], F32, kind="ExternalInput")
    out_yv = nc.dram_tensor("out_yv", [P, YC], F32, kind="ExternalOutput")
    out_stat = nc.dram_tensor("out_stat", [P, 2], F32, kind="ExternalOutput")
    out_d = nc.dram_tensor("out_d", [P, 3], F32, kind="ExternalOutput")
    if debug:
        dbg_muN = nc.dram_tensor("dbg_muN", [P, YC], F32, kind="ExternalOutput")
        dbg_dacc = nc.dram_tensor("dbg_dacc", [P, 4], F32, kind="ExternalOutput")
        dbg_scr = nc.dram_tensor("dbg_scr", [P, P], F32, kind="ExternalOutput")
        dbg_muT = nc.dram_tensor("dbg_muT", [P, 2, P], F32, kind="ExternalOutput")

    ctx = ExitStack()
    with ctx:
        sb = lambda name, shape, dt=F32: ctx.enter_context(
            nc.sbuf_tensor(name, shape, dt)
        )
        ps = lambda name, shape: ctx.enter_context(nc.psum_tensor(name, shape, F32))
        sem = lambda name: ctx.enter_context(nc.semaphore(name))

        xbuf = sb("xbuf", [P, NBUF, CH, S], F16)
        fbuf = sb("fbuf", [P, NF, CH, S // 2], F16)
        xv = sb("xv", [P, XC])
        yv = sb("yv", [P, YC])
        wsb = sb("wsb", [P, WCOLS], F16)
        bsb = sb("bsb", [P, 8])
        xvT = sb("xvT", [P, 4, P], F16)
        hT = sb("hT", [P, 4, P], F16)
        muT = sb("muT", [P, 2, P])
        muN = sb("muN", [P, YC])
        stat2 = sb("stat2", [P, 2])
        dacc = sb("dacc", [P, 4])
        scr = sb("scr", [P, YC])
        ident = sb("ident", [P, P])

        pt = [ps(f"pt{i}", [P, P]) for i in range(2)]
        ph = ps("ph", [P, 4, P])
        pmu = ps("pmu", [P, 2, P])

        dsem = [sem(f"d{u}") for u in range(NU)]
        dw = sem("dw")
        dout = sem("dout")
        s_pool = sem("s_pool")
        s_gfold = sem("s_gfold")
        s_tp = sem("s_tp")
        s_cp = sem("s_cp")
        s_hmm = sem("s_hmm")
        s_relu = sem("s_relu")
        s_mumm = sem("s_mumm")
        s_mucp = sem("s_mucp")
        s_tpmu = sem("s_tpmu")
        s_mun = sem("s_mun")
        s_stat = sem("s_stat")
        s_ttr = sem("s_ttr")
        s_dch = sem("s_dch")

        def chunk_src(c, lo, hi):
            is_y, c0 = CHUNKS[c]
            t = y if is_y else x
            return t[:, c0 + lo:c0 + hi, :]

        def pool_dst(u):
            c, lo, hi = UNITS[u]
            is_y, c0 = CHUNKS[c]
            t = yv if is_y else xv
            return t[:, c0 + lo:c0 + hi]

        with nc.Block() as block:

            @block.sync
            def _(e):
                for u, (c, lo, hi) in enumerate(UNITS):
                    if u == WPOS:
                        e.dma_start(out=wsb[:, :], in_=wpack[:, :]).then_inc(dw, 16)
                        e.dma_start(out=bsb[:, :], in_=bias[:, :]).then_inc(dw, 16)
                        e.dma_start(out=ident[:, :], in_=ident_in[:, :]).then_inc(
                            dw, 16
                        )
                    if c >= NBUF and lo == 0:
                        # ring reuse: chunk c-NBUF fully pooled
                        e.wait_ge(s_pool, ULAST[c - NBUF] + 1)
                    e.dma_start(
                        out=xbuf[:, c % NBUF, lo:hi, :], in_=chunk_src(c, lo, hi)
                    ).then_inc(dsem[u], 16)
                e.wait_ge(dout, 144 if DEBUG else 80)

            @block.gpsimd
            def _(e):
                for u in GUNITS:
                    c, lo, hi = UNITS[u]
                    w = hi - lo
                    e.wait_ge(dsem[u], 16)
                    if u >= NF:
                        e.wait_ge(s_pool, u - NF + 1)
                    e.tensor_add(
                        fbuf[:, u % NF, 0:w, :],
                        xbuf[:, c % NBUF, lo:hi, 0:S // 2],
                        xbuf[:, c % NBUF, lo:hi, S // 2:S],
                    ).then_inc(s_gfold, 1)

            @block.vector
            def _(e):
                def ttr(cols0, cols1, blk):
                    # product into its own scr range; sem-enforce the DVE
                    # write->read hazard (short ops lack the drain interlock)
                    e.tensor_mul(
                        scr[:, cols0:cols1],
                        muN[:, cols0:cols1],
                        yv[:, cols0:cols1],
                    ).then_inc(s_dch, 1)
                    e.wait_ge(s_dch, blk + 1)
                    e.tensor_reduce(
                        dacc[:, blk:blk + 1],
                        scr[:, cols0:cols1],
                        axis=AX.X,
                        op=ALU.add,
                    ).then_inc(s_ttr, 1)

                for u, (c, lo, hi) in enumerate(UNITS):
                    w = hi - lo
                    if u in GRANK:
                        e.wait_ge(s_gfold, GRANK[u] + 1)
                    else:
                        e.wait_ge(dsem[u], 16)
                        e.tensor_add(
                            fbuf[:, u % NF, 0:w, :],
                            xbuf[:, c % NBUF, lo:hi, 0:S // 2],
                            xbuf[:, c % NBUF, lo:hi, S // 2:S],
                        )
                    e.tensor_reduce(
                        pool_dst(u), fbuf[:, u % NF, 0:w, :], axis=AX.X, op=ALU.add
                    ).then_inc(s_pool, 1)
                    if u == ULAST[18]:
                        # Sum_n mu (both halves) once muT is written
                        e.wait_ge(s_mucp, 2)
                        e.tensor_reduce(
                            stat2[:, 0:1], muT[:, 0, :], axis=AX.X, op=ALU.add
                        )
                        e.tensor_reduce(
                            stat2[:, 1:2], muT[:, 1, :], axis=AX.X, op=ALU.add
                        ).then_inc(s_stat, 1)
                    if u == ULAST[19]:
                        e.wait_ge(s_mun, 2)
                        ttr(0, 128, 0)
                    if u == ULAST[22]:
                        ttr(128, 224, 1)
                ttr(224, 256, 2)

            @block.tensor
            def _(e):
                e.wait_ge(dw, 48)
                for m in range(4):
                    e.wait_ge(s_pool, 4 * m + 5)
                    if m >= 2:
                        e.wait_ge(s_cp, m - 1)
                    e.transpose(
                        pt[m % 2][:, :], xv[:, m * P:(m + 1) * P], ident[:, :]
                    ).then_inc(s_tp, 1)
                e.wait_ge(s_cp, 4)
                for m in range(4):
                    for k in range(4):
                        mm = e.matmul(
                            ph[:, m, :],
                            wsb[:, k * HID + m * P:k * HID + (m + 1) * P],
                            xvT[:, k, :],
                            start=(k == 0),
                            stop=(k == 3),
                        )
                mm.then_inc(s_hmm, 1)
                e.wait_ge(s_relu, 4)
                for m in range(2):
                    for k in range(4):
                        mm = e.matmul(
                            pmu[:, m, :],
                            wsb[:, 2048 + k * YC + m * P:2048 + k * YC + (m + 1) * P],
                            hT[:, k, :],
                            start=(k == 0),
                            stop=(k == 3),
                        )
                mm.then_inc(s_mumm, 1)
                e.wait_ge(s_mucp, 2)
                for m in range(2):
                    e.transpose(pt[m][:, :], muT[:, m, :], ident[:, :]).then_inc(
                        s_tpmu, 1
                    )

            @block.scalar
            def _(e):
                for m in range(4):
                    e.wait_ge(s_tp, m + 1)
                    e.activation(
                        xvT[:, m, :], pt[m % 2][:, :], ACTF.Copy, scale=1.0 / S
                    ).then_inc(s_cp, 1)
                e.wait_ge(s_hmm, 1)
                for m in range(4):
                    e.activation(
                        hT[:, m, :], ph[:, m, :], ACTF.Relu, bias=bsb[:, m:m + 1]
                    ).then_inc(s_relu, 1)
                e.wait_ge(s_mumm, 1)
                for m in range(2):
                    e.activation(
                        muT[:, m, :], pmu[:, m, :], ACTF.Identity,
                        bias=bsb[:, 4 + m:5 + m],
                    ).then_inc(s_mucp, 1)
                for m in range(2):
                    e.wait_ge(s_tpmu, m + 1)
                    e.activation(
                        muN[:, m * P:(m + 1) * P], pt[m][:, :], ACTF.Copy
                    ).then_inc(s_mun, 1)
                e.wait_ge(s_stat, 1)
                e.dma_start(out=out_stat[:, :], in_=stat2[:, :]).then_inc(dout, 16)
                e.wait_ge(s_pool, POOL_A)
                e.dma_start(out=out_yv[:, 0:128], in_=yv[:, 0:128]).then_inc(dout, 16)
                e.wait_ge(s_pool, POOL_B)
                e.dma_start(out=out_yv[:, 128:224], in_=yv[:, 128:224]).then_inc(
                    dout, 16
                )
                e.wait_ge(s_pool, POOL_C)
                e.dma_start(out=out_yv[:, 224:256], in_=yv[:, 224:256]).then_inc(
                    dout, 16
                )
                e.wait_ge(s_ttr, 3)
                e.dma_start(out=out_d[:, :], in_=dacc[:, 0:3]).then_inc(dout, 16)
                if debug:
                    e.dma_start(out=dbg_muN[:, :], in_=muN[:, :]).then_inc(dout, 16)
                    e.dma_start(out=dbg_dacc[:, :], in_=dacc[:, :]).then_inc(dout, 16)
                    e.dma_start(out=dbg_scr[:, :], in_=scr[:, :]).then_inc(dout, 16)
                    e.dma_start(out=dbg_muT[:, :, :], in_=muT[:, :, :]).then_inc(
                        dout, 16
                    )

    return nc


def _get_nc():
    if "nc" not in _CACHE:
        _CACHE["nc"] = build_nc(debug=DEBUG)
    return _CACHE["nc"]


def make_in_maps(x_samples, y_samples, W1, b1, W2, b2):
    xs = np.asarray(x_samples, np.float32).reshape(N, XC, S).astype(np.float16)
    ys = np.asarray(y_samples, np.float32).reshape(N, YC, S).astype(np.float16)
    wp = np.zeros((P, WCOLS), np.float16)
    wp[:, :2048] = (
        np.asarray(W1, np.float16).reshape(4, P, HID).transpose(1, 0, 2).reshape(P, 2048)
    )
    wp[:, 2048:3072] = (
        np.asarray(W2, np.float16).reshape(4, P, YC).transpose(1, 0, 2).reshape(P, 1024)
    )
    wp = np.ascontiguousarray(wp)
    bp = np.zeros((P, 8), np.float32)
    bp[:, 0:4] = np.asarray(b1, np.float32).reshape(4, P).T
    bp[:, 4:6] = np.asarray(b2, np.float32).reshape(2, P).T
    bp = np.ascontiguousarray(bp)
    idm = np.ascontiguousarray(np.eye(P, dtype=np.float32))
    in_maps = []
    for c in range(8):
        in_maps.append(
            {
                "x": np.ascontiguousarray(xs[c * P:(c + 1) * P]),
                "y": np.ascontiguousarray(ys[c * P:(c + 1) * P]),
                "wpack": wp,
                "bias": bp,
                "ident_in": idm,
            }
        )
    return in_maps


def combine(results):
    dot = 0.0
    Mu = np.zeros(YC, np.float64)
    Ey = np.zeros(YC, np.float64)
    for c in range(8):
        stat = results[c]["out_stat"].astype(np.float64)   # (128, 2)
        yvc = results[c]["out_yv"].astype(np.float64)      # (128, 256)
        dot += results[c]["out_d"].astype(np.float64).sum()
        Mu += np.concatenate([stat[:, 0], stat[:, 1]])
        Ey += yvc.sum(axis=0)
    dot /= S
    Ey /= S
    loss = dot / N - float((Mu / N) @ (Ey / N))
    return np.float32(loss)


def run(inputs, **kwargs):
    nc = _get_nc()
    in_maps = make_in_maps(**inputs)
    res = run_bass_kernel_spmd(nc, in_maps, core_ids=list(range(8)), **kwargs)
    return combine(res.results), res


def kernel(x_samples, y_samples, W1, b1, W2, b2):
    loss, _ = run(
        dict(
            x_samples=x_samples,
            y_samples=y_samples,
            W1=W1,
            b1=b1,
            W2=W2,
            b2=b2,
        )
    )
    return loss


# revision 13
# speedup vs baseline: 1.2678x; 1.1072x over previous
"""CLUBMean loss kernel for Trainium2, 8-core data-parallel, fp16 stream.

Math: the reference loss collapses exactly (the quadratic terms cancel):
  loss = mean_i mu_i . (y_i - mean_j y_j)
       = (1/N) sum_i mu_i.y_i  -  (sum_i mu_i / N) . (sum_j y_j / N)
so the kernel only needs pooled vectors, the MLP, one covariance dot per
sample, and the two mean vectors. Samples are streamed as fp16 (host cast):
halves HBM traffic; measured end-to-end rel err ~2e-3 vs the 2e-2 gate.

Each core handles 128 of the 1024 samples:
  - sync HWDGE streams x (8 x 64ch) then y (4 x 64ch) fp16 chunks, 1 MiB per
    DMA (8 KiB/partition rows keep the SDMA packets at full efficiency);
    first/last chunks split into sub-DMAs so the pipeline starts/drains fast
  - pooling per unit = fp16 tensor_tensor fold chain 64->32->16->8 (2x DVE
    rate; op count amortized over 64ch) + one f32 tensor_reduce (1x);
    GpSimd takes the level-1 fold on a few x chunks to share load
  - DVE back-to-back ops shorter than the ~420ns pipe-drain window do NOT
    interlock on HW (CoreSim's race detector is right): any such producer->
    consumer edge is semaphore-chained (s_dch)
  - PE transposes pooled x (f32), ACT scale-copies (1/64) to fp16; MLP runs
    as fp16 matmuls into f32 PSUM; mu is back-transposed to sample-major
  - dot blocks D_b = sum_c mu[n,c]*yv[n,c] via sem-chained mul+reduce
  - outputs: yv (pooled y, unscaled; host sums for the y-mean), Mu, D blocks

Host combine (f64): loss = sum(D)/64/N - (Mu/N).(sum(yv)/64/N).
Each DMA's +16 semaphore arrives as +1 per DGE lane; chunk completion uses
one semaphore per transfer.
"""

import sys

sys.path.insert(0, "/opt/trn_rl_repo")

from contextlib import ExitStack

import numpy as np

import concourse.bass as bass
import concourse.mybir as mybir
from concourse.bass_utils import run_bass_kernel_spmd

N = 1024
P = 128            # samples per core
XC, YC, HID, S = 512, 256, 512, 64
CH = 64            # channels per streamed chunk (1 MiB fp16)
NBUF = 8           # stream buffer ring (chunk-indexed)
NF = 4             # fold chain buffer ring (unit-indexed)
WCOLS = 3072       # fp16 weight pack: w1 (4k x 512h) | w2 (4k x 256c)
F32 = mybir.dt.float32
F16 = mybir.dt.float16
AX = mybir.AxisListType
ALU = mybir.AluOpType
ACTF = mybir.ActivationFunctionType

# ---- chunk / pool-unit tables ----------------------------------------------
# chunks: (is_y, c0) with CH channels each. x: 8, y: 4.
CHUNKS = [(0, c * CH) for c in range(8)] + [(1, c * CH) for c in range(4)]
NCHUNK = len(CHUNKS)       # 12
# per-chunk sub-DMA channel ranges (each is one DMA + one pool unit)
SUBS = {0: [(0, 16), (16, 32), (32, 48), (48, 64)],
        11: [(0, 32), (32, 48), (48, 56), (56, 64)]}
WPOS = 5                   # weights ride the stream after this many DMAs
GCHUNKS = (1, 3, 5)        # chunks whose level-1 fold runs on GpSimd

UNITS = []                 # (chunk, lo, hi)
for c in range(NCHUNK):
    for lo, hi in SUBS.get(c, [(0, CH)]):
        UNITS.append((c, lo, hi))
NU = len(UNITS)            # 18
ULAST = {}
for u, (c, lo, hi) in enumerate(UNITS):
    ULAST[c] = u
GUNITS = [u for u, (c, lo, hi) in enumerate(UNITS) if c in GCHUNKS]
GRANK = {u: r for r, u in enumerate(GUNITS)}

# y column blocks for the dot + output DMAs (yv columns)
POOL_A, POOL_B, POOL_C = ULAST[9] + 1, ULAST[10] + 1, ULAST[11] + 1

DEBUG = False

_CACHE = {}


def build_nc(debug=False):
    nc = bass.Bass()
    x = nc.dram_tensor("x", [P, XC, S], F16, kind="ExternalInput")
    y = nc.dram_tensor("y", [P, YC, S], F16, kind="ExternalInput")
    wpack = nc.dram_tensor("wpack", [P, WCOLS], F16, kind="ExternalInput")
    bias = nc.dram_tensor("bias", [P, 8], F32, kind="ExternalInput")
    ident_in = nc.dram_tensor("ident_in", [P, P], F32, kind="ExternalInput")
    out_yv = nc.dram_tensor("out_yv", [P, YC], F32, kind="ExternalOutput")
    out_stat = nc.dram_tensor("out_stat", [P, 2], F32, kind="ExternalOutput")
    out_d = nc.dram_tensor("out_d", [P, 3], F32, kind="ExternalOutput")
    if debug:
        dbg_muN = nc.dram_tensor("dbg_muN", [P, YC], F32, kind="ExternalOutput")
        dbg_dacc = nc.dram_tensor("dbg_dacc", [P, 4], F32, kind="ExternalOutput")

    ctx = ExitStack()
    with ctx:
        sb = lambda name, shape, dt=F32: ctx.enter_context(
            nc.sbuf_tensor(name, shape, dt)
        )
        ps = lambda name, shape: ctx.enter_context(nc.psum_tensor(name, shape, F32))
        sem = lambda name: ctx.enter_context(nc.semaphore(name))

        xbuf = sb("xbuf", [P, NBUF, CH, S], F16)
        f1 = sb("f1", [P, NF, CH, 32], F16)
        f2 = sb("f2", [P, NF, CH, 16], F16)
        f3 = sb("f3", [P, NF, CH, 8], F16)
        xv = sb("xv", [P, XC])
        yv = sb("yv", [P, YC])
        wsb = sb("wsb", [P, WCOLS], F16)
        bsb = sb("bsb", [P, 8])
        xvT = sb("xvT", [P, 4, P], F16)
        hT = sb("hT", [P, 4, P], F16)
        muT = sb("muT", [P, 2, P])
        muN = sb("muN", [P, YC])
        stat2 = sb("stat2", [P, 2])
        dacc = sb("dacc", [P, 4])
        scr = sb("scr", [P, YC])
        ident = sb("ident", [P, P])

        pt = [ps(f"pt{i}", [P, P]) for i in range(2)]
        ph = ps("ph", [P, 4, P])
        pmu = ps("pmu", [P, 2, P])

        dsem = [sem(f"d{u}") for u in range(NU)]
        dw = sem("dw")
        dout = sem("dout")
        s_pool = sem("s_pool")
        s_gfold = sem("s_gfold")
        s_tp = sem("s_tp")
        s_cp = sem("s_cp")
        s_hmm = sem("s_hmm")
        s_relu = sem("s_relu")
        s_mumm = sem("s_mumm")
        s_mucp = sem("s_mucp")
        s_tpmu = sem("s_tpmu")
        s_mun = sem("s_mun")
        s_stat = sem("s_stat")
        s_ttr = sem("s_ttr")
        s_dch = sem("s_dch")

        def chunk_src(c, lo, hi):
            is_y, c0 = CHUNKS[c]
            t = y if is_y else x
            return t[:, c0 + lo:c0 + hi, :]

        def pool_dst(u):
            c, lo, hi = UNITS[u]
            is_y, c0 = CHUNKS[c]
            t = yv if is_y else xv
            return t[:, c0 + lo:c0 + hi]

        with nc.Block() as block:

            @block.sync
            def _(e):
                for u, (c, lo, hi) in enumerate(UNITS):
                    if u == WPOS:
                        e.dma_start(out=wsb[:, :], in_=wpack[:, :]).then_inc(dw, 16)
                        e.dma_start(out=bsb[:, :], in_=bias[:, :]).then_inc(dw, 16)
                        e.dma_start(out=ident[:, :], in_=ident_in[:, :]).then_inc(
                            dw, 16
                        )
                    if c >= NBUF and lo == 0:
                        # ring reuse: chunk c-NBUF fully pooled
                        e.wait_ge(s_pool, ULAST[c - NBUF] + 1)
                    e.dma_start(
                        out=xbuf[:, c % NBUF, lo:hi, :], in_=chunk_src(c, lo, hi)
                    ).then_inc(dsem[u], 16)
                e.wait_ge(dout, 80 + (32 if debug else 0))

            @block.gpsimd
            def _(e):
                for u in GUNITS:
                    c, lo, hi = UNITS[u]
                    e.wait_ge(dsem[u], 16)
                    if u >= NF:
                        e.wait_ge(s_pool, u - NF + 1)
                    e.tensor_add(
                        f1[:, u % NF, 0:hi - lo, :],
                        xbuf[:, c % NBUF, lo:hi, 0:32],
                        xbuf[:, c % NBUF, lo:hi, 32:64],
                    ).then_inc(s_gfold, 1)

            @block.vector
            def _(e):
                dch = [0]

                def chain(inst):
                    # sem-enforce a short-op RAW edge on the DVE
                    dch[0] += 1
                    inst.then_inc(s_dch, 1)
                    e.wait_ge(s_dch, dch[0])

                def pool_unit(u):
                    c, lo, hi = UNITS[u]
                    w = hi - lo
                    s = u % NF
                    if u in GRANK:
                        e.wait_ge(s_gfold, GRANK[u] + 1)
                    else:
                        e.wait_ge(dsem[u], 16)
                        inst = e.tensor_add(
                            f1[:, s, 0:w, :],
                            xbuf[:, c % NBUF, lo:hi, 0:32],
                            xbuf[:, c % NBUF, lo:hi, 32:64],
                        )
                        if w <= 8:
                            # ~300ns op feeding the next: chain the RAW edge
                            chain(inst)
                    if w >= 64:
                        e.tensor_add(
                            f2[:, s, 0:w, :], f1[:, s, 0:w, 0:16],
                            f1[:, s, 0:w, 16:32],
                        )
                        e.tensor_add(
                            f3[:, s, 0:w, :], f2[:, s, 0:w, 0:8],
                            f2[:, s, 0:w, 8:16],
                        )
                        red_in = f3[:, s, 0:w, :]
                    elif w >= 32:
                        e.tensor_add(
                            f2[:, s, 0:w, :], f1[:, s, 0:w, 0:16],
                            f1[:, s, 0:w, 16:32],
                        )
                        red_in = f2[:, s, 0:w, :]
                    else:
                        red_in = f1[:, s, 0:w, :]
                    e.tensor_reduce(
                        pool_dst(u), red_in, axis=AX.X, op=ALU.add
                    ).then_inc(s_pool, 1)

                def dot(blk, cols0, cols1):
                    chain(e.tensor_mul(
                        scr[:, cols0:cols1],
                        muN[:, cols0:cols1],
                        yv[:, cols0:cols1],
                    ))
                    e.tensor_reduce(
                        dacc[:, blk:blk + 1],
                        scr[:, cols0:cols1],
                        axis=AX.X,
                        op=ALU.add,
                    ).then_inc(s_ttr, 1)

                for u in range(NU):
                    pool_unit(u)
                    if u == ULAST[8]:
                        e.wait_ge(s_mucp, 2)
                        e.tensor_reduce(
                            stat2[:, 0:1], muT[:, 0, :], axis=AX.X, op=ALU.add
                        )
                        e.tensor_reduce(
                            stat2[:, 1:2], muT[:, 1, :], axis=AX.X, op=ALU.add
                        ).then_inc(s_stat, 1)
                    if u == ULAST[9]:
                        e.wait_ge(s_mun, 2)
                        dot(0, 0, 128)
                    if u == ULAST[10]:
                        dot(1, 128, 192)
                dot(2, 192, 256)

            @block.tensor
            def _(e):
                e.wait_ge(dw, 48)
                # xv col block m ready after these many pool units
                for m, need in enumerate((5, 7, 9, 11)):
                    e.wait_ge(s_pool, need)
                    if m >= 2:
                        e.wait_ge(s_cp, m - 1)
                    e.transpose(
                        pt[m % 2][:, :], xv[:, m * P:(m + 1) * P], ident[:, :]
                    ).then_inc(s_tp, 1)
                e.wait_ge(s_cp, 4)
                for m in range(4):
                    for k in range(4):
                        mm = e.matmul(
                            ph[:, m, :],
                            wsb[:, k * HID + m * P:k * HID + (m + 1) * P],
                            xvT[:, k, :],
                            start=(k == 0),
                            stop=(k == 3),
                        )
                mm.then_inc(s_hmm, 1)
                e.wait_ge(s_relu, 4)
                for m in range(2):
                    for k in range(4):
                        mm = e.matmul(
                            pmu[:, m, :],
                            wsb[:, 2048 + k * YC + m * P:2048 + k * YC + (m + 1) * P],
                            hT[:, k, :],
                            start=(k == 0),
                            stop=(k == 3),
                        )
                mm.then_inc(s_mumm, 1)
                e.wait_ge(s_mucp, 2)
                for m in range(2):
                    e.transpose(pt[m][:, :], muT[:, m, :], ident[:, :]).then_inc(
                        s_tpmu, 1
                    )

            @block.scalar
            def _(e):
                for m in range(4):
                    e.wait_ge(s_tp, m + 1)
                    e.activation(
                        xvT[:, m, :], pt[m % 2][:, :], ACTF.Copy, scale=1.0 / S
                    ).then_inc(s_cp, 1)
                e.wait_ge(s_hmm, 1)
                for m in range(4):
                    e.activation(
                        hT[:, m, :], ph[:, m, :], ACTF.Relu, bias=bsb[:, m:m + 1]
                    ).then_inc(s_relu, 1)
                e.wait_ge(s_mumm, 1)
                for m in range(2):
                    e.activation(
                        muT[:, m, :], pmu[:, m, :], ACTF.Identity,
                        bias=bsb[:, 4 + m:5 + m],
                    ).then_inc(s_mucp, 1)
                for m in range(2):
                    e.wait_ge(s_tpmu, m + 1)
                    e.activation(
                        muN[:, m * P:(m + 1) * P], pt[m][:, :], ACTF.Copy
                    ).then_inc(s_mun, 1)
                e.wait_ge(s_stat, 1)
                e.dma_start(out=out_stat[:, :], in_=stat2[:, :]).then_inc(dout, 16)
                e.wait_ge(s_pool, POOL_A)
                e.dma_start(out=out_yv[:, 0:128], in_=yv[:, 0:128]).then_inc(dout, 16)
                e.wait_ge(s_pool, POOL_B)
                e.dma_start(out=out_yv[:, 128:192], in_=yv[:, 128:192]).then_inc(
                    dout, 16
                )
                e.wait_ge(s_pool, POOL_C)
                e.dma_start(out=out_yv[:, 192:256], in_=yv[:, 192:256]).then_inc(
                    dout, 16
                )
                e.wait_ge(s_ttr, 3)
                e.dma_start(out=out_d[:, :], in_=dacc[:, 0:3]).then_inc(dout, 16)
                if debug:
                    e.dma_start(out=dbg_muN[:, :], in_=muN[:, :]).then_inc(dout, 16)
                    e.dma_start(out=dbg_dacc[:, :], in_=dacc[:, :]).then_inc(dout, 16)

    return nc


def _get_nc():
    if "nc" not in _CACHE:
        _CACHE["nc"] = build_nc(debug=DEBUG)
    return _CACHE["nc"]


def make_in_maps(x_samples, y_samples, W1, b1, W2, b2):
    xs = np.asarray(x_samples, np.float32).reshape(N, XC, S).astype(np.float16)
    ys = np.asarray(y_samples, np.float32).reshape(N, YC, S).astype(np.float16)
    wp = np.zeros((P, WCOLS), np.float16)
    wp[:, :2048] = (
        np.asarray(W1, np.float16).reshape(4, P, HID).transpose(1, 0, 2).reshape(P, 2048)
    )
    wp[:, 2048:3072] = (
        np.asarray(W2, np.float16).reshape(4, P, YC).transpose(1, 0, 2).reshape(P, 1024)
    )
    wp = np.ascontiguousarray(wp)
    bp = np.zeros((P, 8), np.float32)
    bp[:, 0:4] = np.asarray(b1, np.float32).reshape(4, P).T
    bp[:, 4:6] = np.asarray(b2, np.float32).reshape(2, P).T
    bp = np.ascontiguousarray(bp)
    idm = np.ascontiguousarray(np.eye(P, dtype=np.float32))
    in_maps = []
    for c in range(8):
        in_maps.append(
            {
                "x": np.ascontiguousarray(xs[c * P:(c + 1) * P]),
                "y": np.ascontiguousarray(ys[c * P:(c + 1) * P]),
                "wpack": wp,
                "bias": bp,
                "ident_in": idm,
            }
        )
    return in_maps


def combine(results):
    dot = 0.0
    Mu = np.zeros(YC, np.float64)
    Ey = np.zeros(YC, np.float64)
    for c in range(8):
        stat = results[c]["out_stat"].astype(np.float64)   # (128, 2)
        yvc = results[c]["out_yv"].astype(np.float64)      # (128, 256)
        dot += results[c]["out_d"].astype(np.float64).sum()
        Mu += np.concatenate([stat[:, 0], stat[:, 1]])
        Ey += yvc.sum(axis=0)
    dot /= S
    Ey /= S
    loss = dot / N - float((Mu / N) @ (Ey / N))
    return np.float32(loss)


def run(inputs, **kwargs):
    nc = _get_nc()
    in_maps = make_in_maps(**inputs)
    res = run_bass_kernel_spmd(nc, in_maps, core_ids=list(range(8)), **kwargs)
    return combine(res.results), res


def kernel(x_samples, y_samples, W1, b1, W2, b2):
    loss, _ = run(
        dict(
            x_samples=x_samples,
            y_samples=y_samples,
            W1=W1,
            b1=b1,
            W2=W2,
            b2=b2,
        )
    )
    return loss
